# revision 9
# baseline (speedup 1.0000x reference)
"""Distributed GQA attention layer (16 Q heads / 4 KV heads, RoPE, causal)
for one TRN2 chip (8 NeuronCores), tensor-parallel over heads.

Per core c:
  - 2 Q heads (2c, 2c+1) -> Wq columns [128c, 128c+128)
  - 1 KV head (c//2)     -> Wk/Wv columns [64*(c//2), 64*(c//2)+64)
  - computes O^T for its heads, AllGathers O^T (bf16, 0.5MB/rank),
  - computes Y^T = (O @ Wo[:, cols_c])^T for output columns [128c, 128c+128)
Host concatenates per-core Y^T shards into the full output.

Matmuls run in bf16 (full PE rate, FWL weight loads); all accumulation is
fp32 in PSUM. Layouts keep head_dim on partitions (Q^T/K^T): scores are
built transposed (S^T tiles [k,q]), softmax along the free q axis needs no
partition reductions — exp on ACT, per-q sums via a ones-column fused into
V (extra PSUM row), causal masking via affine_select on gpsimd.
"""

import numpy as np
import ml_dtypes

import concourse.bass as bass
import concourse.bacc as bacc
import concourse.mybir as mybir
from concourse import tile
from concourse.bass_utils import run_bass_kernel_spmd
from concourse.masks import make_identity

N_CORES = 8
SEQ = 2048
EMB = 1024
HD = 64  # head dim
ROPE_BASE = 10000.0

F32 = mybir.dt.float32
BF16 = mybir.dt.bfloat16

NKT = SEQ // 128  # 16 seq k-tiles
NCH = SEQ // 512  # 4 seq chunks of 512
NET = EMB // 128  # 8 embedding k-tiles


def build_nc():
    nc = bacc.Bacc("TRN2", target_bir_lowering=False, debug=False,
                   num_devices=N_CORES)

    xt_d = nc.declare_dram_parameter("xt", [EMB, SEQ], BF16, isOutput=False)
    wq_d = nc.declare_dram_parameter("wq", [EMB, 128], BF16, isOutput=False)
    wk_d = nc.declare_dram_parameter("wk", [EMB, HD], BF16, isOutput=False)
    wv_d = nc.declare_dram_parameter("wv", [EMB, HD], BF16, isOutput=False)
    wo_d = nc.declare_dram_parameter("wo", [EMB, 128], BF16, isOutput=False)
    cos_d = nc.declare_dram_parameter("cos2", [128, SEQ], F32, isOutput=False)
    sin_d = nc.declare_dram_parameter("sin2", [128, SEQ], F32, isOutput=False)
    out_d = nc.declare_dram_parameter("out", [128, SEQ], F32, isOutput=True)

    with tile.TileContext(nc, num_cores=N_CORES) as tc:
        with (
            tc.tile_pool(name="const", bufs=1) as constp,
            tc.tile_pool(name="qkv_sb", bufs=1) as qkvp,
            tc.tile_pool(name="ot_sb", bufs=1) as otp,
            tc.tile_pool(name="dram", bufs=1, space="DRAM") as dramp,
        ):
            # ---- constants -------------------------------------------------
            c2 = constp.tile([128, SEQ], F32, name="c2")
            s2 = constp.tile([128, SEQ], F32, name="s2")
            nc.sync.dma_start(c2[:], cos_d[:])
            nc.sync.dma_start(s2[:], sin_d[:])
            ident = constp.tile([HD, HD], BF16, name="ident")
            make_identity(nc, ident[:])

            # persistent QKV tensors (per 512-chunk tiles for fine deps)
            qt0 = [qkvp.tile([128, 512], BF16, name=f"qt0_{c}") for c in range(NCH)]
            qt1 = [qkvp.tile([HD, 512], BF16, name=f"qt1_{c}") for c in range(NCH)]
            kt = [qkvp.tile([HD, 512], BF16, name=f"kt_{c}") for c in range(NCH)]
            vt = [qkvp.tile([HD, 512], BF16, name=f"vt_{c}") for c in range(NCH)]
            vn = [qkvp.tile([128, HD + 1], BF16, name=f"vn_{k}") for k in range(NKT)]
            ot = [[otp.tile([HD, 512], BF16, name=f"ot_{h}_{c}") for c in range(NCH)]
                  for h in range(2)]

            # ---- QKV projection + rope + V transpose ----------------------
            with (
                tc.tile_pool(name="wpool", bufs=1) as wp,
                tc.tile_pool(name="xtp", bufs=1) as xtp,
                tc.tile_pool(name="pj_ps", bufs=6, space="PSUM") as pjps,
                tc.tile_pool(name="rope_tmp", bufs=4) as rtp,
                tc.tile_pool(name="vt_ps", bufs=2, space="PSUM") as vtps,
            ):
                wq_sb = wp.tile([128, NET * 128], BF16, name="wq_sb")
                wk_sb = wp.tile([128, NET * HD], BF16, name="wk_sb")
                wv_sb = wp.tile([128, NET * HD], BF16, name="wv_sb")
                nc.sync.dma_start(
                    wq_sb[:].rearrange("p (k m) -> p k m", k=NET),
                    wq_d[:].rearrange("(k p) m -> p k m", p=128))
                nc.sync.dma_start(
                    wk_sb[:].rearrange("p (k m) -> p k m", k=NET),
                    wk_d[:].rearrange("(k p) m -> p k m", p=128))
                nc.sync.dma_start(
                    wv_sb[:].rearrange("p (k m) -> p k m", k=NET),
                    wv_d[:].rearrange("(k p) m -> p k m", p=128))

                xts = []
                for k in range(NET):
                    xt_sb = xtp.tile([128, SEQ], BF16, name=f"xt_sb{k}")
                    nc.sync.dma_start(xt_sb[:], xt_d[128 * k:128 * (k + 1), :])
                    xts.append(xt_sb)

                # psum [*,512] per chunk, accumulate over emb tiles
                psq = [pjps.tile([128, 512], F32, name=f"psq{c}", tag="pj")
                       for c in range(NCH)]
                for k in range(NET):
                    for c in range(NCH):
                        nc.tensor.matmul(
                            psq[c][:],
                            wq_sb[:, 128 * k:128 * (k + 1)],
                            xts[k][:, 512 * c:512 * (c + 1)],
                            start=(k == 0), stop=(k == NET - 1))
                psk = [pjps.tile([HD, 512], F32, name=f"psk{c}", tag="pj")
                       for c in range(NCH)]
                for k in range(NET):
                    for c in range(NCH):
                        nc.tensor.matmul(
                            psk[c][:],
                            wk_sb[:, HD * k:HD * (k + 1)],
                            xts[k][:, 512 * c:512 * (c + 1)],
                            start=(k == 0), stop=(k == NET - 1))
                psv = [pjps.tile([HD, 512], F32, name=f"psv{c}", tag="pj")
                       for c in range(NCH)]
                for k in range(NET):
                    for c in range(NCH):
                        nc.tensor.matmul(
                            psv[c][:],
                            wv_sb[:, HD * k:HD * (k + 1)],
                            xts[k][:, 512 * c:512 * (c + 1)],
                            start=(k == 0), stop=(k == NET - 1))

                # rope on Q (both heads at once, [128,512] chunks)
                half = 32
                for c in range(NCH):
                    sl = slice(512 * c, 512 * (c + 1))
                    t1 = rtp.tile([128, 512], F32, name="t1", tag="t1")
                    nc.vector.tensor_mul(t1[:], psq[c][:], c2[:, sl])
                    rsw = rtp.tile([128, 512], F32, name="rsw", tag="rsw")
                    # rotate_half per 64-row head block (swap 32-row halves)
                    nc.scalar.copy(rsw[0:32, :], psq[c][32:64, :])
                    nc.scalar.copy(rsw[32:64, :], psq[c][0:32, :])
                    nc.scalar.copy(rsw[64:96, :], psq[c][96:128, :])
                    nc.scalar.copy(rsw[96:128, :], psq[c][64:96, :])
                    t2 = rtp.tile([128, 512], F32, name="t2", tag="t2")
                    nc.vector.tensor_mul(t2[:], rsw[:], s2[:, sl])
                    nc.vector.tensor_add(qt0[c][:], t1[:], t2[:])
                    # head-1 copy to partition base 0
                    nc.scalar.copy(qt1[c][:], qt0[c][HD:2 * HD, :])

                # rope on K ([64,512] chunks)
                for c in range(NCH):
                    sl = slice(512 * c, 512 * (c + 1))
                    t1k = rtp.tile([HD, 512], F32, name="t1k", tag="t1")
                    nc.vector.tensor_mul(t1k[:], psk[c][:], c2[0:HD, sl])
                    rswk = rtp.tile([HD, 512], F32, name="rswk", tag="rsw")
                    nc.scalar.copy(rswk[0:half, :], psk[c][half:2 * half, :])
                    nc.scalar.copy(rswk[half:2 * half, :], psk[c][0:half, :])
                    t2k = rtp.tile([HD, 512], F32, name="t2k", tag="t2")
                    nc.vector.tensor_mul(t2k[:], rswk[:], s2[0:HD, sl])
                    nc.vector.tensor_add(kt[c][:], t1k[:], t2k[:])
                    # V: plain copy psum -> sbuf (converts to bf16)
                    nc.vector.tensor_copy(vt[c][:], psv[c][:])

                # V transpose -> natural [seq,hd] tiles with ones column
                for k in range(NKT):
                    c, j = k // 4, k % 4
                    pv = vtps.tile([128, HD], BF16, name="pv", tag="pv")
                    nc.tensor.transpose(
                        pv[:], vt[c][:, 128 * j:128 * (j + 1)], ident[:])
                    nc.scalar.copy(vn[k][:, 0:HD], pv[:])
                    nc.gpsimd.memset(vn[k][:, HD:HD + 1], 1.0)

            # ---- attention per head ---------------------------------------
            with (
                tc.tile_pool(name="s_ps", bufs=3, space="PSUM") as sps,
                tc.tile_pool(name="e_sb", bufs=3) as esb,
                tc.tile_pool(name="o_ps", bufs=2, space="PSUM") as ops,
                tc.tile_pool(name="r_sb", bufs=2) as rsb,
            ):
                for h in range(2):
                    q_chunks = qt0 if h == 0 else qt1
                    for qc in range(NCH):
                        qb = 512 * qc
                        qtile = q_chunks[qc]
                        qap = qtile[0:HD, :] if h == 0 else qtile[:]
                        pso = ops.tile([HD + 1, 512], F32, name="pso", tag="pso")
                        kts = list(range(4 * qc + 4))
                        for ki in kts:
                            c, j = ki // 4, ki % 4
                            pss = sps.tile([128, 512], F32, name="pss", tag="pss")
                            nc.tensor.matmul(
                                pss[:],
                                kt[c][:, 128 * j:128 * (j + 1)],
                                qap,
                                start=True, stop=True)
                            e = esb.tile([128, 512], BF16, name="e", tag="e")
                            nc.scalar.activation(
                                e[:], pss[:],
                                mybir.ActivationFunctionType.Exp, scale=0.125)
                            if 128 * ki + 127 > qb:  # boundary tile: causal mask
                                nc.gpsimd.affine_select(
                                    out=e[:], in_=e[:],
                                    compare_op=mybir.AluOpType.is_ge,
                                    fill=0.0,
                                    base=qb - 128 * ki,
                                    channel_multiplier=-1,
                                    pattern=[[1, 512]])
                            nc.tensor.matmul(
                                pso[:], vn[ki][:], e[:],
                                start=(ki == kts[0]), stop=(ki == kts[-1]))
                        # normalize: rows 0..63 = O^T, row 64 = sums
                        rec = rsb.tile([1, 512], F32, name="rec", tag="rec")
                        nc.vector.reciprocal(rec[:], pso[HD:HD + 1, :])
                        bc = rsb.tile([HD, 512], F32, name="bc", tag="bc")
                        nc.gpsimd.partition_broadcast(bc[:], rec[:])
                        nc.vector.tensor_mul(ot[h][qc][:], pso[0:HD, :], bc[:])

            # ---- AllGather O^T across the 8 cores -------------------------
            ag_in = dramp.tile([128, SEQ], BF16, name="ag_in")
            ag_out = dramp.tile([N_CORES * 128, SEQ], BF16, name="ag_out",
                                addr_space="Shared")
            for h in range(2):
                for c in range(NCH):
                    nc.sync.dma_start(
                        ag_in[HD * h:HD * (h + 1), 512 * c:512 * (c + 1)],
                        ot[h][c][:])
            nc.gpsimd.collective_compute(
                "AllGather",
                mybir.AluOpType.bypass,
                ins=[ag_in[:]],
                outs=[ag_out[:]],
                replica_groups=[list(range(N_CORES))],
            )

            # ---- final projection: Y^T[:,128 cols shard] ------------------
            with (
                tc.tile_pool(name="fin_sb", bufs=1) as fsb,
                tc.tile_pool(name="y_ps", bufs=1, space="PSUM") as yps,
            ):
                wo_sb = fsb.tile([128, NET * 128], BF16, name="wo_sb")
                nc.sync.dma_start(
                    wo_sb[:].rearrange("p (k m) -> p k m", k=NET),
                    wo_d[:].rearrange("(k p) m -> p k m", p=128))
                yt = fsb.tile([128, SEQ], F32, name="yt")
                psy = [yps.tile([128, 512], F32, name=f"psy{c}") for c in range(NCH)]
                otf = []
                for k in range(NET):
                    of = fsb.tile([128, SEQ], BF16, name=f"otf{k}")
                    nc.sync.dma_start(of[:], ag_out[128 * k:128 * (k + 1), :])
                    otf.append(of)
                for k in range(NET):
                    for c in range(NCH):
                        nc.tensor.matmul(
                            psy[c][:],
                            wo_sb[:, 128 * k:128 * (k + 1)],
                            otf[k][:, 512 * c:512 * (c + 1)],
                            start=(k == 0), stop=(k == NET - 1))
                for c in range(NCH):
                    nc.vector.tensor_copy(yt[:, 512 * c:512 * (c + 1)], psy[c][:])
                nc.sync.dma_start(out_d[:], yt[:])

    nc.finalize()
    return nc


def _rope_tables():
    inv_freq = 1.0 / (ROPE_BASE ** (np.arange(0, HD, 2, dtype=np.float32) / HD))
    t = np.arange(SEQ, dtype=np.float32)
    freqs = np.outer(t, inv_freq)  # [seq, 32]
    cos = np.cos(freqs).T.astype(np.float32)  # [32, seq]
    sin = np.sin(freqs).T.astype(np.float32)
    c_blk = np.concatenate([cos, cos], axis=0)          # [64, seq]
    s_blk = np.concatenate([-sin, sin], axis=0)         # [64, seq]
    c2 = np.concatenate([c_blk, c_blk], axis=0)         # [128, seq]
    s2 = np.concatenate([s_blk, s_blk], axis=0)
    return np.ascontiguousarray(c2), np.ascontiguousarray(s2)


def _shard_inputs(hidden_states, Wq, Wk, Wv, Wo):
    bf = ml_dtypes.bfloat16
    xt = np.ascontiguousarray(hidden_states[0].T).astype(bf)  # [emb, seq]
    c2, s2 = _rope_tables()
    in_maps = []
    for c in range(N_CORES):
        kv = c // 2
        in_maps.append({
            "xt": xt,
            "wq": np.ascontiguousarray(Wq[:, 128 * c:128 * (c + 1)]).astype(bf),
            "wk": np.ascontiguousarray(Wk[:, HD * kv:HD * (kv + 1)]).astype(bf),
            "wv": np.ascontiguousarray(Wv[:, HD * kv:HD * (kv + 1)]).astype(bf),
            "wo": np.ascontiguousarray(Wo[:, 128 * c:128 * (c + 1)]).astype(bf),
            "cos2": c2,
            "sin2": s2,
        })
    return in_maps


_NC_CACHE = {}


def kernel(hidden_states, Wq, Wk, Wv, Wo):
    hidden_states = np.asarray(hidden_states, dtype=np.float32)
    Wq = np.asarray(Wq, dtype=np.float32)
    Wk = np.asarray(Wk, dtype=np.float32)
    Wv = np.asarray(Wv, dtype=np.float32)
    Wo = np.asarray(Wo, dtype=np.float32)

    b, s, e = hidden_states.shape
    assert (b, s, e) == (1, SEQ, EMB)

    if "nc" not in _NC_CACHE:
        _NC_CACHE["nc"] = build_nc()
    nc = _NC_CACHE["nc"]

    in_maps = _shard_inputs(hidden_states, Wq, Wk, Wv, Wo)
    res = run_bass_kernel_spmd(nc, in_maps, core_ids=list(range(N_CORES)))
    cols = [np.asarray(r["out"]).T for r in res.results]  # each [seq, 128]
    out = np.concatenate(cols, axis=1)  # [seq, emb]
    return out.reshape(1, SEQ, EMB).astype(np.float32)
